# revision 10
# baseline (speedup 1.0000x reference)
"""TP-8 LMAttention prefill kernel for Trainium2 (Bass/Tile).

Sharding: core c owns q-heads 4c..4c+3 and kv-head c; x replicated.
wo input-dim shard => each core returns a partial [3072, 2048] output,
summed on host.

Dataflow is feature-major ("everything transposed") so no on-chip
transposes are needed:
  xT [D, T] (host-pretransposed, bf16)
  qT/kT = wT.T @ xT         -> [hd, t]
  v     = xT_tile.T @ wvT   -> [t, hd]  (natural layout for AV lhsT)
  ST    = kT_tile.T @ qT    -> [tk, tq] scores, exp'd via ACT (scale folded)
  causal mask: affine_select fill=0 post-exp on diagonal tiles
  rowsum l = ones[128,1].T @ expT (PSUM-accumulated over tk tiles)
  yT    = v_tile.T @ expT   -> [hd, tq], normalized by 1/l (partition-bcast)
  oT    = woT_tile.T @ yT   -> [dim, tq] partial output

RoPE (interleaved) is done with a half-swap permutation of the hd axis
(host permutes wq/wk rows and freq tables; even dims -> partitions 0..63,
odd dims -> 64..127) so the pairwise rotate becomes two 64-partition
shifted multiplies; signs folded into the FS table.
"""

import numpy as np
import ml_dtypes

T = 2048
D = 3072
HD = 128
NB = 4          # tq blocks of 512
TQB = 512
KT = 24         # d-tiles of 128 in D
NCORES = 8
SCALE = 1.0 / float(np.sqrt(HD))

_BF16 = ml_dtypes.bfloat16

_nc_cache = {}


def _build_nc():
    """Build the per-core Bass program (identical on all 8 cores)."""
    import concourse.bacc as bacc
    import concourse.tile as tile
    import concourse.mybir as mybir

    f32 = mybir.dt.float32
    bf16 = mybir.dt.bfloat16

    nc = bacc.Bacc("TRN2", target_bir_lowering=False, debug=False)

    xT = nc.dram_tensor("xt", [D, T], bf16, kind="ExternalInput")
    wq = nc.dram_tensor("wqt", [D, 4 * HD], bf16, kind="ExternalInput")
    wk = nc.dram_tensor("wkt", [D, HD], bf16, kind="ExternalInput")
    wv = nc.dram_tensor("wvt", [D, HD], bf16, kind="ExternalInput")
    wo = nc.dram_tensor("wot", [4 * HD, D], bf16, kind="ExternalInput")
    fc = nc.dram_tensor("fc", [HD, T], f32, kind="ExternalInput")
    fs = nc.dram_tensor("fs", [HD, T], f32, kind="ExternalInput")
    out = nc.dram_tensor("out", [D, T], f32, kind="ExternalOutput")

    with tile.TileContext(nc) as tc:
        import contextlib

        ctx = contextlib.ExitStack()
        with ctx:
            wpool = ctx.enter_context(tc.tile_pool(name="weights", bufs=1))
            xpool = ctx.enter_context(tc.tile_pool(name="xblk", bufs=2))
            kvpool = ctx.enter_context(tc.tile_pool(name="kv", bufs=1))
            qpool = ctx.enter_context(tc.tile_pool(name="q", bufs=2))
            tpool = ctx.enter_context(tc.tile_pool(name="tmp", bufs=2))
            epool = ctx.enter_context(tc.tile_pool(name="exp", bufs=4))
            ypool = ctx.enter_context(tc.tile_pool(name="y", bufs=5))
            opool = ctx.enter_context(tc.tile_pool(name="ostage", bufs=2))
            rpool = ctx.enter_context(tc.tile_pool(name="r", bufs=2))
            pp_big = ctx.enter_context(
                tc.tile_pool(name="pbig", bufs=3, space="PSUM"))
            pp_l = ctx.enter_context(
                tc.tile_pool(name="pl", bufs=1, space="PSUM"))
            pp_y = ctx.enter_context(
                tc.tile_pool(name="py", bufs=2, space="PSUM"))
            pp_o = ctx.enter_context(
                tc.tile_pool(name="po", bufs=2, space="PSUM"))

            # ---- persistent weights / tables ----
            wq_sb = wpool.tile([128, KT * 4 * 128], bf16)
            nc.sync.dma_start(
                out=wq_sb.rearrange("p (kt m) -> p kt m", kt=KT),
                in_=wq.rearrange("(kt p) m -> p kt m", p=128))
            wk_sb = wpool.tile([128, KT * 128], bf16)
            nc.sync.dma_start(
                out=wk_sb.rearrange("p (kt m) -> p kt m", kt=KT),
                in_=wk.rearrange("(kt p) m -> p kt m", p=128))
            wv_sb = wpool.tile([128, KT * 128], bf16)
            nc.sync.dma_start(
                out=wv_sb.rearrange("p (kt m) -> p kt m", kt=KT),
                in_=wv.rearrange("(kt p) m -> p kt m", p=128))
            wo_sb = wpool.tile([128, 4 * D], bf16)
            nc.scalar.dma_start(
                out=wo_sb.rearrange("p (h m) -> p h m", h=4),
                in_=wo.rearrange("(h p) m -> p h m", p=128))
            fc_sb = wpool.tile([128, T], f32)
            nc.scalar.dma_start(out=fc_sb, in_=fc[:, :])
            fs_sb = wpool.tile([128, T], f32)
            nc.scalar.dma_start(out=fs_sb, in_=fs[:, :])
            ones_sb = wpool.tile([128, 1], bf16)
            nc.vector.memset(ones_sb, 1.0)
            masks = []
            for o in range(4):
                mk = wpool.tile([128, TQB], bf16, name=f"mask{o}")
                nc.gpsimd.memset(mk, 1.0)
                nc.gpsimd.affine_select(
                    out=mk, in_=mk, pattern=[[1, TQB]],
                    compare_op=mybir.AluOpType.is_ge, fill=0.0,
                    base=-(o * 128), channel_multiplier=-1)
                masks.append(mk)

            # persistent K^T [hd, T] and V-natural [t, hd] (both bf16)
            kT_sb = kvpool.tile([128, T], bf16)
            v_sb = kvpool.tile([128, 16 * 128], bf16)

            xTr = xT.rearrange("(kt p) t -> p kt t", p=128)

            for b in range(NB):
                ts = slice(b * TQB, (b + 1) * TQB)
                x_blk = xpool.tile([128, KT * TQB], bf16)
                nc.sync.dma_start(
                    out=x_blk.rearrange("p (kt t) -> p kt t", kt=KT),
                    in_=xTr[:, :, ts])
                xb = x_blk.rearrange("p (kt t) -> p kt t", kt=KT)

                q_sb = qpool.tile([128, 4 * TQB], bf16)

                # ---- q/k projections + RoPE ----
                for h in range(5):  # 0..3 = q heads, 4 = k
                    pq = pp_big.tile([128, TQB], mybir.dt.float32, tag="big")
                    for kt in range(KT):
                        if h < 4:
                            lhs = wq_sb[:, kt * 512 + h * 128:
                                        kt * 512 + (h + 1) * 128]
                        else:
                            lhs = wk_sb[:, kt * 128:(kt + 1) * 128]
                        nc.tensor.matmul(pq, lhs, xb[:, kt, :],
                                         start=(kt == 0), stop=(kt == KT - 1))
                    # RoPE: out = pq*FC + swap64(pq)*FS  (cast to bf16)
                    t1 = tpool.tile([128, TQB], mybir.dt.float32, tag="t1")
                    nc.vector.tensor_tensor(t1, pq, fc_sb[:, ts],
                                            mybir.AluOpType.mult)
                    t2 = tpool.tile([128, TQB], mybir.dt.float32, tag="t2")
                    nc.vector.tensor_tensor(t2[0:64, :], pq[64:128, :],
                                            fs_sb[0:64, ts],
                                            mybir.AluOpType.mult)
                    nc.vector.tensor_tensor(t2[64:128, :], pq[0:64, :],
                                            fs_sb[64:128, ts],
                                            mybir.AluOpType.mult)
                    dst = (q_sb[:, h * TQB:(h + 1) * TQB] if h < 4
                           else kT_sb[:, ts])
                    nc.vector.tensor_tensor(dst, t1, t2, mybir.AluOpType.add)

                # ---- v projection (natural layout) ----
                for tt in range(4):
                    pv = pp_big.tile([128, 128], mybir.dt.float32, tag="big")
                    for kt in range(KT):
                        nc.tensor.matmul(
                            pv,
                            xb[:, kt, tt * 128:(tt + 1) * 128],
                            wv_sb[:, kt * 128:(kt + 1) * 128],
                            start=(kt == 0), stop=(kt == KT - 1))
                    nc.vector.tensor_copy(
                        v_sb[:, (b * 4 + tt) * 128:(b * 4 + tt + 1) * 128],
                        pv)

                # ---- attention, head-outer ----
                ntk = 4 * (b + 1)
                ybs = []
                for h in range(4):
                    py = pp_y.tile([128, TQB], mybir.dt.float32)
                    pl = pp_l.tile([1, TQB], mybir.dt.float32)
                    for j in range(ntk):
                        ps = pp_big.tile([128, TQB], mybir.dt.float32,
                                         tag="big")
                        nc.tensor.matmul(
                            ps, kT_sb[:, j * 128:(j + 1) * 128],
                            q_sb[:, h * TQB:(h + 1) * TQB],
                            start=True, stop=True)
                        e = epool.tile([128, TQB], mybir.dt.bfloat16)
                        nc.scalar.activation(
                            e, ps, mybir.ActivationFunctionType.Exp,
                            scale=SCALE)
                        if j >= 4 * b:  # diagonal tile -> causal mask
                            nc.vector.tensor_tensor(
                                e, e, masks[j - 4 * b],
                                mybir.AluOpType.mult)
                        nc.tensor.matmul(
                            py, v_sb[:, j * 128:(j + 1) * 128], e,
                            start=(j == 0), stop=(j == ntk - 1))
                        nc.tensor.matmul(
                            pl, ones_sb, e,
                            start=(j == 0), stop=(j == ntk - 1))
                    linv = rpool.tile([1, TQB], mybir.dt.float32, tag="linv")
                    nc.vector.reciprocal(linv, pl)
                    lb = rpool.tile([128, TQB], mybir.dt.float32, tag="lb")
                    nc.gpsimd.partition_broadcast(lb, linv)
                    yb = ypool.tile([128, TQB], mybir.dt.bfloat16)
                    nc.vector.tensor_tensor(yb, py, lb, mybir.AluOpType.mult)
                    ybs.append(yb)

                # ---- output projection (partial over this core's heads) ----
                for dt in range(KT):
                    po = pp_o.tile([128, TQB], mybir.dt.float32)
                    for h in range(4):
                        nc.tensor.matmul(
                            po,
                            wo_sb[:, h * D + dt * 128:h * D + (dt + 1) * 128],
                            ybs[h],
                            start=(h == 0), stop=(h == 3))
                    ot = opool.tile([128, TQB], mybir.dt.float32)
                    nc.vector.tensor_copy(ot, po)
                    nc.sync.dma_start(
                        out=out[dt * 128:(dt + 1) * 128, ts], in_=ot)

    nc.compile()
    return nc


def _get_nc():
    if "nc" not in _nc_cache:
        _nc_cache["nc"] = _build_nc()
    return _nc_cache["nc"]


def _prep_inputs(x, wq, wk, wv, wo, freqs_cos, freqs_sin):
    """Host-side shard + layout prep. Returns in_maps for 8 cores."""
    x2 = np.asarray(x, np.float32).reshape(T, D)
    xT = np.ascontiguousarray(x2.T).astype(_BF16)

    perm = np.concatenate([np.arange(0, HD, 2), np.arange(1, HD, 2)])
    sign = np.ones(HD, np.float32)
    sign[:64] = -1.0

    fcT = np.ascontiguousarray(
        np.asarray(freqs_cos, np.float32)[:, perm].T)        # [128, T]
    fsT = np.ascontiguousarray(
        (np.asarray(freqs_sin, np.float32)[:, perm] * sign[None, :]).T)

    wq = np.asarray(wq, np.float32)
    wk = np.asarray(wk, np.float32)
    wv = np.asarray(wv, np.float32)
    wo = np.asarray(wo, np.float32)

    in_maps = []
    for c in range(NCORES):
        wq_c = wq[c * 512:(c + 1) * 512].reshape(4, HD, D)[:, perm, :]
        wq_c = wq_c.reshape(4 * HD, D)
        wk_c = wk[c * HD:(c + 1) * HD][perm, :]
        wv_c = wv[c * HD:(c + 1) * HD]
        wo_c = wo[:, c * 512:(c + 1) * 512]
        in_maps.append({
            "xt": xT,
            "wqt": np.ascontiguousarray(wq_c.T).astype(_BF16),
            "wkt": np.ascontiguousarray(wk_c.T).astype(_BF16),
            "wvt": np.ascontiguousarray(wv_c.T).astype(_BF16),
            "wot": np.ascontiguousarray(wo_c.T).astype(_BF16),
            "fc": fcT,
            "fs": fsT,
        })
    return in_maps


def run(x, wq, wk, wv, wo, freqs_cos, freqs_sin, trace=False, **_):
    from concourse import bass_utils
    nc = _get_nc()
    in_maps = _prep_inputs(x, wq, wk, wv, wo, freqs_cos, freqs_sin)
    kw = {}
    if trace:
        kw = dict(trace=True, trace_cores=[0])
    res = bass_utils.run_bass_kernel_spmd(
        nc, in_maps, core_ids=list(range(NCORES)), **kw)
    acc = np.zeros((D, T), np.float64)
    for r in res.results:
        acc += r["out"].astype(np.float64)
    out = np.ascontiguousarray(acc.T.astype(np.float32)).reshape(1, T, D)
    return out, res


def kernel(x, wq, wk, wv, wo, freqs_cos, freqs_sin,
           k_cache=None, v_cache=None, input_pos=None, **_):
    # input_pos is always 0 and the caches are zero-filled; every cache
    # position >= T is causally masked for all queries, so the caches
    # never contribute to the output.
    out, _res = run(x, wq, wk, wv, wo, freqs_cos, freqs_sin, trace=False)
    return out
